# revision 32
# baseline (speedup 1.0000x reference)
"""ALiBi sliding-window multi-head attention on 8 TRN2 NeuronCores.

Sharding: head-parallel. Core i computes heads (i, i+8) — one
large-slope head in slot 0 (computes only diagonal bands d in {0,1,2};
the dropped bands' weights are <= 1.1e-7 for every head 0..7) and one
small-slope head in slot 1 (keeps all 5 bands of the 512-wide window).
Full inputs are prepared host-side (x transposed + bf16-cast, w_kqv
column slices per core) and each core's [4096, 128] output slab is
re-interleaved into the [1, 4096, 1024] result.

Device kernel per core:
  - projection: K^T,Q^T in [head_dim, seq] layout (2 heads stacked on
    128 partitions), V in natural [seq, head_dim] layout with an
    appended ones column (gives the softmax denominator for free in
    the attention*V matmul).
  - attention over the causal 512-window: per (query-block, diag d)
    score tile S^T = K_blk^T{}^T @ Q_blk^T on TensorE, then ScalarE
    exp(S/32 + m*(jj-63-128d)) with a per-partition bias vector.  The
    query-side ALiBi factor exp(m*(ii-63)) is constant per output
    column across all d and cancels in the normalization, so it is
    never computed.  d=0 / d=4 tiles get a 0/1 mask multiply on DVE.
  - out[q, :] = (sum_d P_d^T @ [V|1]) scaled by the reciprocal of its
    ones-column, per-partition on DVE.
"""

import sys

sys.path.insert(0, "/opt/trn_rl_repo")

import numpy as np
import ml_dtypes

import concourse.bass as bass
import concourse.mybir as mybir
import concourse.tile as tile
from concourse.bass import broadcast_tensor_aps
from concourse.bass_utils import run_bass_kernel_spmd
from concourse.vector_clock import ScopedClock, VectorClock

BF16 = ml_dtypes.bfloat16
BF = mybir.dt.bfloat16
F32 = mybir.dt.float32

S, D, H, HD, WINDOW = 4096, 1024, 16, 64, 512
NCORES = 8
NQB = S // 128          # 32 query blocks
NSC = S // 512          # 8 seq chunks
INV_SCALE = 1.0 / 32.0  # 1/sqrt(d_model)


def _patch_tile_drain():
    """Chunk the TileContext final-drain sem waits across single-wait nops
    (walrus rejects instructions carrying many sync waits)."""
    if getattr(tile.TileContext, "_drain_chunk_patched", False):
        return

    def _drain_and_barrier_chunked(self, tick_clock, wait_clock):
        gc = tick_clock.global_clock
        nprocs = len(gc)
        cur = ScopedClock({None: VectorClock([0] * nprocs)})
        for p in range(nprocs):
            t = gc[p]
            if t > 0:
                sc = ScopedClock({None: VectorClock([0] * nprocs)})
                sc.require_at_least(None, p, t)
                nop_inst = self.nc.sync.nop(hint="drain_wait_chunk", nofuse=True)
                wait_clock.add_sem_waits(nop_inst.ins, sc, cur)
                cur.update_past(sc)
        drain_inst = self.nc.sync.drain()
        wait_clock.add_sem_waits(
            drain_inst.ins, ScopedClock({None: tick_clock.global_clock}), cur
        )
        self.nc.all_engine_barrier()
        assert self.sems is not None
        popped = self.nc._tile_sem_poison_stack.pop()
        assert popped is self._sem_poison
        # the walrus NEFF epilogue re-zeroes the full semaphore file after
        # this point, so skip clear_and_free_semaphores + second barrier
        self.nc._state.prepend_free_semaphores(
            [s.num if hasattr(s, "num") else s
             for s in self.sems.allocated().values()]
        )

    tile.TileContext._drain_and_barrier = _drain_and_barrier_chunked
    tile.TileContext._drain_chunk_patched = True


def _split_multi_waits(nc):
    """This walrus build rejects instructions carrying more than one sync
    wait; hoist excess waits onto same-engine NoOps inserted just before."""
    def walk(block):
        insts = block.instructions
        i = 0
        while i < len(insts):
            inst = insts[i]
            si = inst.sync_info
            if si is not None and si.on_wait and len(si.on_wait) > 1:
                waits = list(si.on_wait)
                for w in waits[:-1]:
                    nop = mybir.InstNoOp(
                        name=nc.get_next_instruction_name(),
                        engine=inst.engine,
                        sync_info=mybir.SyncInfo(on_wait=[w], on_update=[]),
                        bass_nofuse=True,
                    )
                    nc.register_instruction(nop)
                    insts.insert(i, nop)
                    i += 1
                inst.sync_info = mybir.SyncInfo(
                    on_wait=waits[-1:], on_update=list(si.on_update or [])
                )
            i += 1
        for b in getattr(block, "blocks", None) or []:
            walk(b)

    for b in nc.m.functions[0].blocks:
        walk(b)


def build_graph():
    _patch_tile_drain()
    nc = bass.Bass()
    xT_d = nc.declare_dram_parameter("xT", [NSC, 8, 128, 512], BF, isOutput=False)
    wk_d = nc.declare_dram_parameter("wk", [128, 8, 128], BF, isOutput=False)
    wq_d = nc.declare_dram_parameter("wq", [128, 8, 128], BF, isOutput=False)
    wv_d = nc.declare_dram_parameter("wv", [128, 8, 128], BF, isOutput=False)
    bias_d = nc.declare_dram_parameter("bias", [128, 10], F32, isOutput=False)
    mask_d = nc.declare_dram_parameter("mask", [128, 256], F32, isOutput=False)
    out_d = nc.declare_dram_parameter("out", [S, 128], F32, isOutput=True)

    Exp = mybir.ActivationFunctionType.Exp
    MUL = mybir.AluOpType.mult

    with tile.TileContext(nc) as tc:
        with (
            tc.tile_pool(name="const", bufs=1) as const,
            tc.tile_pool(name="big", bufs=1) as big,
            tc.tile_pool(name="P", bufs=28) as ppool,
            tc.tile_pool(name="ptmp", bufs=4) as tmpp,
            tc.tile_pool(name="outsb", bufs=6) as outp,
            tc.tile_pool(name="recp", bufs=4) as recp,
            tc.tile_pool(name="kqps", bufs=2, space="PSUM") as kqps,
            tc.tile_pool(name="vps", bufs=1, space="PSUM") as vps,
            tc.tile_pool(name="sps", bufs=3, space="PSUM") as sps,
            tc.tile_pool(name="ops", bufs=2, space="PSUM") as ops,
        ):
            w_sb = {}
            for name, wd in (("k", wk_d), ("q", wq_d), ("v", wv_d)):
                w_sb[name] = const.tile([128, 8, 128], BF, tag=f"w{name}", name=f"w{name}_sb")
                nc.sync.dma_start(w_sb[name][:], wd[:])
            bias_sb = const.tile([128, 10], F32, name="bias_sb")
            mask_sb = const.tile([128, 256], F32, name="mask_sb")

            xT_sb = big.tile([128, 8, S], BF, tag="xT", name="xT_sb")
            kT_sb = big.tile([128, S], BF, tag="kT", name="kT_sb")
            qT_sb = big.tile([128, S], BF, tag="qT", name="qT_sb")
            # V_aug: per seq-tile t and head slot h, 65 cols at (t*2+h)*65
            v_sb = big.tile([128, 2 * NQB, 65], BF, tag="v", name="v_sb")
            nc.vector.memset(v_sb[:, :, 64:65], 1.0)

            def emit_dma_in(sc):
                s0 = sc * 512
                for c in range(8):
                    nc.sync.dma_start(
                        xT_sb[:, c, s0:s0 + 512], xT_d[sc, c]
                    )

            def emit_proj(sc):
                s0 = sc * 512
                accs = {}
                for wname in ("k", "q"):
                    accs[wname] = kqps.tile([128, 512], F32, tag="kq", name="kq_acc")
                for c in range(8):
                    for wname in ("k", "q"):
                        nc.tensor.matmul(
                            accs[wname][:],
                            w_sb[wname][:, c, :],
                            xT_sb[:, c, s0:s0 + 512],
                            start=(c == 0),
                            stop=(c == 7),
                        )
                for wname, dstT in (("k", kT_sb), ("q", qT_sb)):
                    nc.vector.tensor_copy(dstT[:, s0:s0 + 512], accs[wname][:])
                vacc = vps.tile([128, 512], F32, tag="v", name="v_acc")
                for t4 in range(4):
                    st = sc * 4 + t4
                    p0 = st * 128
                    for c in range(8):
                        nc.tensor.matmul(
                            vacc[:, t4 * 128:t4 * 128 + 128],
                            xT_sb[:, c, p0:p0 + 128],
                            w_sb["v"][:, c, :],
                            start=(c == 0),
                            stop=(c == 7),
                        )
                nc.vector.tensor_copy(
                    v_sb[:, sc * 8:sc * 8 + 8, 0:64],
                    vacc.rearrange("p (s h x) -> p (s h) x", h=2, x=64),
                )

            def emit_scores(sc):
                """QK + exp + mask for the 4 query blocks of chunk sc.
                Returns {(hl, d): P tile [128, 512] bf16}.  The two head
                slots' QK matmuls are emitted pairwise adjacent: slot 1
                lives on PE row groups 2-3 and slot 0 on 0-1, so the pair
                executes concurrently on disjoint sub-arrays."""
                qbs = [sc * 4 + i for i in range(4)]
                ptiles = {}
                for d in range(5):
                    hls = [hl for hl in (1, 0) if d < (3 if hl == 0 else 5)]
                    valid = [qb for qb in qbs if qb - d >= 0]
                    if not valid:
                        continue
                    sp = {}
                    for hl in hls:
                        sp[hl] = sps.tile([128, 512], F32, tag="s", name="s_psum")
                    for qb in valid:
                        kb = qb - d
                        off = (qb % 4) * 128
                        for hl in hls:
                            nc.tensor.matmul(
                                sp[hl][:, off:off + 128],
                                kT_sb[hl * 64:hl * 64 + 64, kb * 128:kb * 128 + 128],
                                qT_sb[hl * 64:hl * 64 + 64, qb * 128:qb * 128 + 128],
                                start=True,
                                stop=True,
                            )
                    lo = (valid[0] % 4) * 128
                    hi = (valid[-1] % 4) * 128 + 128
                    for hl in hls:
                        P = ppool.tile([128, 512], BF, tag="P", name="P_sb")
                        b_ap = bias_sb[:, hl * 5 + d:hl * 5 + d + 1]
                        if d == 0 or d == 4:
                            pt = tmpp.tile([128, 512], F32, tag="ptmp", name="p_tmp")
                            nc.scalar.activation(
                                pt[:, lo:hi], sp[hl][:, lo:hi], Exp,
                                bias=b_ap, scale=INV_SCALE,
                            )
                            mi = 0 if d == 0 else 1
                            in0 = pt[:, lo:hi].rearrange("p (g i) -> p g i", i=128)
                            pout = P[:, lo:hi].rearrange("p (g i) -> p g i", i=128)
                            mask3 = mask_sb.rearrange("p (m i) -> p m i", i=128)
                            in0b, in1b = broadcast_tensor_aps(
                                in0, mask3[:, mi:mi + 1, :]
                            )
                            nc.vector.tensor_tensor(pout, in0b, in1b, op=MUL)
                        else:
                            nc.scalar.activation(
                                P[:, lo:hi], sp[hl][:, lo:hi], Exp,
                                bias=b_ap, scale=INV_SCALE,
                            )
                        ptiles[(hl, d)] = P
                return ptiles

            def emit_av(sc, ptiles):
                qbs = [sc * 4 + i for i in range(4)]
                # last chunk: per-qb output tiles/DMAs so the final store
                # overlaps the remaining epilogues instead of trailing them
                grp = 1 if sc == NSC - 1 else 4
                for qb in qbs:
                    if qb % grp == 0:
                        ot = outp.tile([128, grp, 128], F32, tag="o", name="out_tile")
                    off = (qb % 4) * 128
                    # both heads in one PSUM bank: [h0 out|denom | h1 out|denom]
                    oacc = ops.tile([128, 130], F32, tag="oacc", name="o_acc")
                    for hl in (1, 0):
                        ndmax = 3 if hl == 0 else 5
                        ds_ = [d for d in range(ndmax) if qb - d >= 0]
                        for j, d in enumerate(ds_):
                            kb = qb - d
                            nc.tensor.matmul(
                                oacc[:, hl * 65:hl * 65 + 65],
                                ptiles[(hl, d)][:, off:off + 128],
                                v_sb[:, kb * 2 + hl, :],
                                start=(j == 0),
                                stop=(j == len(ds_) - 1),
                            )
                    oacc3 = oacc.rearrange("p (h x) -> p h x", x=65)
                    rec = recp.tile([128, 2, 1], F32, tag="rec", name="rec")
                    nc.vector.reciprocal(rec[:], oacc3[:, :, 64:65])
                    otq = ot[:, qb % grp, :].rearrange("p (h x) -> p h x", x=64)
                    in0b, in1b = broadcast_tensor_aps(oacc3[:, :, 0:64], rec[:])
                    nc.vector.tensor_tensor(otq, in0b, in1b, op=MUL)
                    if qb % grp == grp - 1:
                        q0 = (qb - grp + 1) * 128
                        nc.scalar.dma_start(
                            out_d[q0:q0 + grp * 128, :].rearrange(
                                "(q p) ch -> p q ch", p=128
                            ),
                            ot[:],
                        )

            # HAM warm-up: ~3.5us of throwaway matmuls on never-written
            # SBUF while the first xT chunk DMAs in, so real matmuls start
            # at 2.4 GHz instead of 1.2.
            warm_in = outp.tile([128, 512], BF, tag="warm", name="warm_in")
            nc.vector.memset(warm_in[:], 1.0)
            for wi in range(8):
                wacc = kqps.tile([128, 512], F32, tag="kq", name="warm_acc")
                nc.tensor.matmul(
                    wacc[:], warm_in[:, 0:128], warm_in[:], start=True, stop=True
                )

            emit_dma_in(0)
            nc.sync.dma_start(bias_sb[:], bias_d[:])
            nc.sync.dma_start(mask_sb[:], mask_d[:])
            emit_proj(0)

            # software pipeline: proj(it) fills TensorE while ScalarE runs
            # chunk it-1's exps; AV lags two chunks so its P tiles are
            # always ready when TensorE reaches them.
            pending = {}
            for it in range(1, NSC + 2):
                if it <= NSC:
                    cur = emit_scores(it - 1)
                    pending[it - 1] = cur
                if it < NSC:
                    emit_dma_in(it)
                    emit_proj(it)
                if it - 2 in pending:
                    emit_av(it - 2, pending.pop(it - 2))
            assert not pending

    _split_multi_waits(nc)
    return nc


_GRAPH = None


def _get_graph():
    global _GRAPH
    if _GRAPH is None:
        _GRAPH = build_graph()
    return _GRAPH


def kernel(x: np.ndarray, w_kqv: np.ndarray) -> np.ndarray:
    x = np.asarray(x, dtype=np.float32)
    w = np.asarray(w_kqv, dtype=np.float32)
    assert x.shape == (1, S, D) and w.shape == (D, 3 * D)

    slopes = [(2.0 ** 8) ** (-(h + 1) / 16.0) for h in range(H)]
    xT = np.ascontiguousarray(x[0].T).astype(BF16)  # [1024, 4096]
    # [sc, c, p, s] — each (chunk, contraction-block) slab contiguous
    xT_bf = np.ascontiguousarray(xT.reshape(8, 128, 8, 512).transpose(2, 0, 1, 3))

    jj = np.arange(128, dtype=np.float32)
    ii = np.arange(128, dtype=np.float32)
    masks = np.zeros((128, 256), np.float32)
    masks[:, 0:128] = (jj[:, None] <= ii[None, :]).astype(np.float32)
    masks[:, 128:256] = (ii[None, :] <= jj[:, None]).astype(np.float32)

    in_maps = []
    for i in range(NCORES):
        heads = (i, i + 8)
        cs = lambda h: slice(h * 64, (h + 1) * 64)
        def wslice(base):
            ws = np.concatenate(
                [w[:, base * 1024:][:, cs(h)] for h in heads], axis=1
            ).astype(BF16)  # [1024, 128]
            # device layout: [p, c, o] with c = contraction block
            return np.ascontiguousarray(ws.reshape(8, 128, 128).transpose(1, 0, 2))
        wk, wq, wv = wslice(0), wslice(1), wslice(2)
        bias = np.zeros((128, 10), np.float32)
        for hl, h in enumerate(heads):
            for d in range(5):
                bias[:, hl * 5 + d] = slopes[h] * (jj - 63.0 - 128.0 * d)
        in_maps.append(
            {"xT": xT_bf, "wk": wk, "wq": wq, "wv": wv, "bias": bias, "mask": masks}
        )

    nc = _get_graph()
    res = run_bass_kernel_spmd(nc, in_maps, core_ids=list(range(NCORES)))

    out_full = np.zeros((1, S, D), np.float32)
    for i in range(NCORES):
        r = np.asarray(res.results[i]["out"], dtype=np.float32)
        out_full[0, :, i * 64:(i + 1) * 64] = r[:, 0:64]
        out_full[0, :, (i + 8) * 64:(i + 9) * 64] = r[:, 64:128]
    return out_full
